# revision 2
# baseline (speedup 1.0000x reference)
"""Trainium2 Bass kernel for nn_MoEBlock (top-1 MoE, E=8 experts).

Strategy (8 NeuronCores):
  Launch 1 (gate, token-parallel): each core computes fp32 gating for its
    512-token shard: red = x @ wr.T; logits = red @ normalize(wg).T;
    top-1 score (1/sum(exp(l - max))) and argmax index.
  Host: all-to-all dispatch — gather each expert's tokens (transposed,
    padded to CAP=768) for its owning core.
  Launch 2 (FFN, expert-parallel): core c holds expert c's weights in
    fp32r (TF32-like, full PE rate at N>=256): hT = gelu(W1.T @ xgT + b1),
    outT = (W2.T @ hT + b2) * score. Padded slots carry score=0 so they
    contribute exactly zero. Per-core partial sums reduce to the scalar
    total on-device.
  Host: scatter rows back by token id, add 8 partial sums.

Hardcoded for B=2, T=2048, C=1024, H=4096, E=8 (fixed problem shapes).
"""
import os
import sys

for _p in ("/root/.axon_site/_ro/trn_rl_repo", "/opt/trn_rl_repo"):
    if os.path.isdir(_p) and _p not in sys.path:
        sys.path.append(_p)

import numpy as np

import concourse.bacc as bacc
import concourse.mybir as mybir
import concourse.tile as tile
from concourse.bass_utils import run_bass_kernel_spmd

F32 = mybir.dt.float32
F32R = mybir.dt.float32r
I32 = mybir.dt.int32
U32 = mybir.dt.uint32
AF = mybir.ActivationFunctionType
ALU = mybir.AluOpType

S = 4096          # tokens
C = 1024          # model dim
H = 4096          # ffn dim
E = 8             # experts
RED = 16          # gate reduction dim
NCORES = 8
SHARD = S // NCORES   # tokens per core in the gate kernel
CAP = 768             # max tokens routed to one expert (actual max is 725)

# module-level cache: compiled programs + last exec times
_cache = {}
last_exec_ns = {"gate": None, "ffn": None}


def _trace_flag():
    return bool(int(os.environ.get("MOE_TRACE", "0")))


# --------------------------------------------------------------------------
# Launch 1: gating
# --------------------------------------------------------------------------
def _build_gate():
    nc = bacc.Bacc("TRN2", target_bir_lowering=False, debug=False,
                   num_devices=NCORES)
    KC = C // 128  # 8 K-chunks

    xt_d = nc.dram_tensor("xt", [C, SHARD], F32, kind="ExternalInput").ap()
    wrt_d = nc.dram_tensor("wrt", [C, RED], F32, kind="ExternalInput").ap()
    wg_d = nc.dram_tensor("wg", [E, RED], F32, kind="ExternalInput").ap()
    iden_d = nc.dram_tensor("iden", [E, E], F32, kind="ExternalInput").ap()
    score_d = nc.dram_tensor("score", [SHARD], F32, kind="ExternalOutput").ap()
    idx_d = nc.dram_tensor("idx", [SHARD], I32, kind="ExternalOutput").ap()

    with tile.TileContext(nc) as tc:
        with tc.tile_pool(name="sb", bufs=1) as sb, \
             tc.tile_pool(name="sm", bufs=2) as sm, \
             tc.tile_pool(name="ps", bufs=2, space="PSUM") as ps, \
             tc.tile_pool(name="psl", bufs=2, space="PSUM") as psl:
            xt_sb = sb.tile([128, KC * SHARD], F32, name="xt_sb")
            nc.sync.dma_start(
                out=xt_sb.rearrange("p (k n) -> p k n", k=KC),
                in_=xt_d.rearrange("(k p) n -> p k n", p=128))
            wrt_sb = sb.tile([128, KC * RED], F32, name="wrt_sb")
            nc.sync.dma_start(
                out=wrt_sb.rearrange("p (k n) -> p k n", k=KC),
                in_=wrt_d.rearrange("(k p) n -> p k n", p=128))
            wg_sb = sb.tile([E, RED], F32, name="wg_sb")
            nc.sync.dma_start(out=wg_sb, in_=wg_d)
            iden_sb = sb.tile([E, E], F32, name="iden_sb")
            nc.sync.dma_start(out=iden_sb, in_=iden_d)

            # --- normalize wg rows: wg / max(||wg||, 1e-4) ---
            sq = sm.tile([E, RED], F32, name="sq")
            nc.vector.tensor_tensor(out=sq, in0=wg_sb, in1=wg_sb, op=ALU.mult)
            nrm = sm.tile([E, 1], F32, name="nrm")
            nc.vector.tensor_reduce(out=nrm, in_=sq, axis=mybir.AxisListType.X,
                                    op=ALU.add)
            nc.scalar.sqrt(nrm, nrm)
            nc.vector.tensor_scalar_max(out=nrm, in0=nrm, scalar1=1e-4)
            rcp = sm.tile([E, 1], F32, name="rcp")
            nc.vector.reciprocal(rcp, nrm)
            wgn = sm.tile([E, RED], F32, name="wgn")
            nc.vector.tensor_scalar_mul(out=wgn, in0=wg_sb, scalar1=rcp)

            # --- transpose wgn -> [RED, E] via PE ---
            pt = ps.tile([RED, E], F32, name="pt")
            nc.tensor.transpose(pt, wgn, iden_sb)
            wgnt = sm.tile([RED, E], F32, name="wgnt")
            nc.vector.tensor_copy(wgnt, pt)

            # --- redT [16, SHARD] = wrT.T @ xT  (fp32 exact) ---
            pr = ps.tile([RED, SHARD], F32, name="pr")
            for k in range(KC):
                nc.tensor.matmul(pr, wrt_sb[:, k * RED:(k + 1) * RED],
                                 xt_sb[:, k * SHARD:(k + 1) * SHARD],
                                 start=(k == 0), stop=(k == KC - 1))
            red_sb = sb.tile([RED, SHARD], F32, name="red_sb")
            nc.vector.tensor_copy(red_sb, pr)

            # --- per 128-token chunk: logits, score, argmax ---
            nchunk = SHARD // 128
            score_sb = sb.tile([128, nchunk], F32, name="score_sb")
            idx_sb = sb.tile([128, nchunk], I32, name="idx_sb")
            for j in range(nchunk):
                pl = psl.tile([128, E], F32, name="pl")
                nc.tensor.matmul(pl, red_sb[:, j * 128:(j + 1) * 128], wgnt,
                                 start=True, stop=True)
                lg = sm.tile([128, E], F32, name="lg")
                nc.vector.tensor_copy(lg, pl)
                mx8 = sm.tile([128, 8], F32, name="mx8")
                nc.vector.max(mx8, lg)
                ix8 = sm.tile([128, 8], U32, name="ix8")
                nc.vector.max_index(ix8, mx8, lg)
                nc.vector.tensor_copy(idx_sb[:, j:j + 1], ix8[:, 0:1])
                neg = sm.tile([128, 1], F32, name="neg")
                nc.vector.tensor_scalar_mul(out=neg, in0=mx8[:, 0:1],
                                            scalar1=-1.0)
                ex = sm.tile([128, E], F32, name="ex")
                ssum = sm.tile([128, 1], F32, name="ssum")
                nc.scalar.activation(ex, lg, AF.Exp, bias=neg, accum_out=ssum)
                nc.vector.reciprocal(score_sb[:, j:j + 1], ssum)

            nc.sync.dma_start(
                out=score_d.rearrange("(j p) -> p j", p=128), in_=score_sb)
            nc.sync.dma_start(
                out=idx_d.rearrange("(j p) -> p j", p=128), in_=idx_sb)

    nc.compile()
    return nc


# --------------------------------------------------------------------------
# Launch 2: expert FFN
# --------------------------------------------------------------------------
def _build_ffn():
    nc = bacc.Bacc("TRN2", target_bir_lowering=False, debug=False,
                   num_devices=NCORES)
    KC = C // 128    # 8
    MC = H // 128    # 32
    HB = 4           # H-chunks per w1 stream block
    NB = H // (HB * 128)  # 8 w1 stream blocks
    N0 = 512         # moving-dim split of CAP: 512 + 256
    N1 = CAP - N0

    xgt_d = nc.dram_tensor("xgt", [C, CAP], F32R, kind="ExternalInput").ap()
    w1_d = nc.dram_tensor("w1", [C, H], F32R, kind="ExternalInput").ap()
    w2_d = nc.dram_tensor("w2", [H, C], F32R, kind="ExternalInput").ap()
    b1_d = nc.dram_tensor("b1", [H], F32, kind="ExternalInput").ap()
    b2_d = nc.dram_tensor("b2", [C], F32, kind="ExternalInput").ap()
    sc_d = nc.dram_tensor("sc", [CAP], F32, kind="ExternalInput").ap()
    out_d = nc.dram_tensor("out", [C, CAP], F32, kind="ExternalOutput").ap()
    tot_d = nc.dram_tensor("tot", [1, 1], F32, kind="ExternalOutput").ap()

    with tile.TileContext(nc) as tc:
        with tc.tile_pool(name="cst", bufs=1) as cst, \
             tc.tile_pool(name="xg", bufs=1) as xg, \
             tc.tile_pool(name="hh", bufs=1) as hh, \
             tc.tile_pool(name="ps", bufs=3, space="PSUM") as ps, \
             tc.tile_pool(name="pss", bufs=1, space="PSUM") as pss, \
             tc.tile_pool(name="ot", bufs=3) as ot:
            b1_sb = cst.tile([128, MC], F32, name="b1_sb")
            nc.sync.dma_start(out=b1_sb, in_=b1_d.rearrange("(m p) -> p m", p=128))
            b2_sb = cst.tile([128, KC], F32, name="b2_sb")
            nc.sync.dma_start(out=b2_sb, in_=b2_d.rearrange("(m p) -> p m", p=128))
            sc_row = cst.tile([1, CAP], F32, name="sc_row")
            nc.sync.dma_start(out=sc_row, in_=sc_d[None, :])
            ones_row = cst.tile([1, 128], F32, name="ones_row")
            nc.vector.memset(ones_row, 1.0)
            ones_col = cst.tile([128, 1], F32, name="ones_col")
            nc.vector.memset(ones_col, 1.0)

            # broadcast scores across partitions via K=1 matmul
            psb = ps.tile([128, CAP], F32, name="p")
            nc.tensor.matmul(psb[:, 0:N0], ones_row, sc_row[:, 0:N0],
                             start=True, stop=True)
            nc.tensor.matmul(psb[:, N0:CAP], ones_row, sc_row[:, N0:CAP],
                             start=True, stop=True)
            scb = cst.tile([128, CAP], F32, name="scb")
            nc.vector.tensor_copy(scb, psb)

            xg_sb = xg.tile([128, KC * CAP], F32R, name="xg_sb")
            nc.sync.dma_start(
                out=xg_sb.rearrange("p (k n) -> p k n", k=KC),
                in_=xgt_d.rearrange("(k p) n -> p k n", p=128))
            h_sb = hh.tile([128, MC * CAP], F32R, name="h_sb")

            # ---- fc1: hT = gelu(W1.T @ xgT + b1) ----
            with tc.tile_pool(name="w1p", bufs=2) as w1p:
                for mb in range(NB):
                    w1blk = w1p.tile([128, KC * HB * 128], F32R, name="w1blk")
                    nc.sync.dma_start(
                        out=w1blk.rearrange("p (k n) -> p k n", k=KC),
                        in_=w1_d[:, mb * HB * 128:(mb + 1) * HB * 128]
                            .rearrange("(k p) n -> p k n", p=128))
                    for mj in range(HB):
                        m = mb * HB + mj
                        p = ps.tile([128, CAP], F32, name="p")
                        for k in range(KC):
                            lhsT = w1blk[:, k * HB * 128 + mj * 128:
                                         k * HB * 128 + (mj + 1) * 128]
                            nc.tensor.matmul(p[:, 0:N0], lhsT,
                                             xg_sb[:, k * CAP:k * CAP + N0],
                                             start=(k == 0), stop=(k == KC - 1))
                            nc.tensor.matmul(p[:, N0:CAP], lhsT,
                                             xg_sb[:, k * CAP + N0:(k + 1) * CAP],
                                             start=(k == 0), stop=(k == KC - 1))
                        nc.scalar.activation(h_sb[:, m * CAP:(m + 1) * CAP], p,
                                             AF.Gelu, bias=b1_sb[:, m:m + 1])

            # ---- fc2: outT = (W2.T @ hT + b2) * score ----
            part_sb = cst.tile([128, KC], F32, name="part_sb")
            with tc.tile_pool(name="w2p", bufs=3) as w2p:
                for m in range(KC):
                    w2blk = w2p.tile([128, MC * 128], F32R, name="w2blk")
                    nc.sync.dma_start(
                        out=w2blk.rearrange("p (k n) -> p k n", k=MC),
                        in_=w2_d[:, m * 128:(m + 1) * 128]
                            .rearrange("(k p) n -> p k n", p=128))
                    p = ps.tile([128, CAP], F32, name="p")
                    for k in range(MC):
                        lhsT = w2blk[:, k * 128:(k + 1) * 128]
                        nc.tensor.matmul(p[:, 0:N0], lhsT,
                                         h_sb[:, k * CAP:k * CAP + N0],
                                         start=(k == 0), stop=(k == MC - 1))
                        nc.tensor.matmul(p[:, N0:CAP], lhsT,
                                         h_sb[:, k * CAP + N0:(k + 1) * CAP],
                                         start=(k == 0), stop=(k == MC - 1))
                    o = ot.tile([128, CAP], F32, name="o")
                    nc.vector.scalar_tensor_tensor(
                        out=o, in0=p, scalar=b2_sb[:, m:m + 1], in1=scb,
                        op0=ALU.add, op1=ALU.mult,
                        accum_out=part_sb[:, m:m + 1])
                    nc.sync.dma_start(out=out_d[m * 128:(m + 1) * 128, :], in_=o)

            # ---- total = sum of partials ----
            rsum = cst.tile([128, 1], F32, name="rsum")
            nc.vector.tensor_reduce(out=rsum, in_=part_sb,
                                    axis=mybir.AxisListType.X, op=ALU.add)
            ptot = pss.tile([1, 1], F32, name="ptot")
            nc.tensor.matmul(ptot, ones_col, rsum, start=True, stop=True)
            tot_sb = cst.tile([1, 1], F32, name="tot_sb")
            nc.vector.tensor_copy(tot_sb, ptot)
            nc.sync.dma_start(out=tot_d, in_=tot_sb)

    nc.compile()
    return nc


def _get(name, builder):
    if name not in _cache:
        _cache[name] = builder()
    return _cache[name]


def _run(nc, in_maps, tag):
    trace = _trace_flag()
    if trace:
        res = run_bass_kernel_spmd(nc, in_maps, core_ids=list(range(NCORES)),
                                   trace=True)
        last_exec_ns[tag] = res.exec_time_ns
        return res.results
    res = run_bass_kernel_spmd(nc, in_maps, core_ids=list(range(NCORES)))
    return res.results


# --------------------------------------------------------------------------
# Host orchestration
# --------------------------------------------------------------------------
def kernel(x, wr, wg, w1, b1, w2, b2):
    x = np.ascontiguousarray(np.asarray(x, dtype=np.float32))
    wr = np.ascontiguousarray(np.asarray(wr, dtype=np.float32))
    wg = np.ascontiguousarray(np.asarray(wg, dtype=np.float32))
    w1 = np.ascontiguousarray(np.asarray(w1, dtype=np.float32))
    b1 = np.ascontiguousarray(np.asarray(b1, dtype=np.float32))
    w2 = np.ascontiguousarray(np.asarray(w2, dtype=np.float32))
    b2 = np.ascontiguousarray(np.asarray(b2, dtype=np.float32))

    B, T, _ = x.shape
    xf = x.reshape(S, C)
    xT = np.ascontiguousarray(xf.T)            # [C, S]
    wrt = np.ascontiguousarray(wr.T)           # [C, RED]
    iden = np.eye(E, dtype=np.float32)

    # ---- launch 1: gating (token-parallel shards) ----
    gate_nc = _get("gate", _build_gate)
    in_maps = [{
        "xt": np.ascontiguousarray(xT[:, c * SHARD:(c + 1) * SHARD]),
        "wrt": wrt, "wg": wg, "iden": iden,
    } for c in range(NCORES)]
    gres = _run(gate_nc, in_maps, "gate")
    idx = np.concatenate([gres[c]["idx"] for c in range(NCORES)])
    score = np.concatenate([gres[c]["score"] for c in range(NCORES)])

    # ---- host all-to-all dispatch ----
    token_lists = []
    ffn_maps = []
    for c in range(NCORES):
        tok = np.nonzero(idx == c)[0].astype(np.int64)
        n = tok.size
        assert n <= CAP, f"expert {c} got {n} tokens > CAP={CAP}"
        token_lists.append(tok)
        xgt = np.zeros((C, CAP), dtype=np.float32)
        xgt[:, :n] = xT[:, tok]
        sc = np.zeros(CAP, dtype=np.float32)
        sc[:n] = score[tok]
        ffn_maps.append({
            "xgt": xgt, "w1": w1[c], "w2": w2[c],
            "b1": b1[c], "b2": b2[c], "sc": sc,
        })

    # ---- launch 2: expert FFN ----
    ffn_nc = _get("ffn", _build_ffn)
    fres = _run(ffn_nc, ffn_maps, "ffn")

    # ---- combine ----
    out = np.empty((S, C), dtype=np.float32)
    total = np.float32(0.0)
    for c in range(NCORES):
        tok = token_lists[c]
        out[tok] = fres[c]["out"][:, :tok.size].T
        total = np.float32(total + fres[c]["tot"][0, 0])
    return out.reshape(B, T, C), total


# revision 4
# speedup vs baseline: 1.0080x; 1.0080x over previous
"""Trainium2 Bass kernel for nn_MoEBlock (top-1 MoE, E=8 experts).

Strategy (8 NeuronCores):
  Launch 1 (gate, token-parallel): each core computes fp32 gating for its
    512-token shard: red = x @ wr.T; logits = red @ normalize(wg).T;
    top-1 score (1/sum(exp(l - max))) and argmax index.
  Host: all-to-all dispatch — gather each expert's tokens (transposed,
    padded to CAP=768) for its owning core.
  Launch 2 (FFN, expert-parallel): core c holds expert c's weights in
    fp32r (TF32-like, full PE rate at N>=256): hT = gelu(W1.T @ xgT + b1),
    outT = (W2.T @ hT + b2) * score. Padded slots carry score=0 so they
    contribute exactly zero. Per-core partial sums reduce to the scalar
    total on-device.
  Host: scatter rows back by token id, add 8 partial sums.

Hardcoded for B=2, T=2048, C=1024, H=4096, E=8 (fixed problem shapes).
"""
import os
import sys

for _p in ("/root/.axon_site/_ro/trn_rl_repo", "/opt/trn_rl_repo"):
    if os.path.isdir(_p) and _p not in sys.path:
        sys.path.append(_p)

import numpy as np

import concourse.bacc as bacc
import concourse.mybir as mybir
import concourse.tile as tile
from concourse.bass_utils import run_bass_kernel_spmd

F32 = mybir.dt.float32
F32R = mybir.dt.float32r
I32 = mybir.dt.int32
U32 = mybir.dt.uint32
AF = mybir.ActivationFunctionType
ALU = mybir.AluOpType

S = 4096          # tokens
C = 1024          # model dim
H = 4096          # ffn dim
E = 8             # experts
RED = 16          # gate reduction dim
NCORES = 8
SHARD = S // NCORES   # tokens per core in the gate kernel
CAP = 768             # max tokens routed to one expert (actual max is 725)

# module-level cache: compiled programs + last exec times
_cache = {}
last_exec_ns = {"gate": None, "ffn": None}


def _trace_flag():
    return bool(int(os.environ.get("MOE_TRACE", "0")))


# --------------------------------------------------------------------------
# Launch 1: gating
# --------------------------------------------------------------------------
def _build_gate():
    nc = bacc.Bacc("TRN2", target_bir_lowering=False, debug=False,
                   num_devices=NCORES)
    KC = C // 128  # 8 K-chunks

    xt_d = nc.dram_tensor("xt", [C, SHARD], F32, kind="ExternalInput").ap()
    wrt_d = nc.dram_tensor("wrt", [C, RED], F32, kind="ExternalInput").ap()
    wg_d = nc.dram_tensor("wg", [E, RED], F32, kind="ExternalInput").ap()
    iden_d = nc.dram_tensor("iden", [E, E], F32, kind="ExternalInput").ap()
    score_d = nc.dram_tensor("score", [SHARD], F32, kind="ExternalOutput").ap()
    idx_d = nc.dram_tensor("idx", [SHARD], I32, kind="ExternalOutput").ap()

    with tile.TileContext(nc) as tc:
        with tc.tile_pool(name="sb", bufs=1) as sb, \
             tc.tile_pool(name="sm", bufs=2) as sm, \
             tc.tile_pool(name="ps", bufs=2, space="PSUM") as ps, \
             tc.tile_pool(name="psl", bufs=2, space="PSUM") as psl:
            wrt_sb = sb.tile([128, KC * RED], F32, name="wrt_sb")
            nc.gpsimd.dma_start(
                out=wrt_sb.rearrange("p (k n) -> p k n", k=KC),
                in_=wrt_d.rearrange("(k p) n -> p k n", p=128))
            xt_sb = sb.tile([128, KC * SHARD], F32, name="xt_sb")
            for k in range(KC):
                nc.sync.dma_start(
                    out=xt_sb[:, k * SHARD:(k + 1) * SHARD],
                    in_=xt_d[k * 128:(k + 1) * 128, :])
            wg_sb = sb.tile([E, RED], F32, name="wg_sb")
            nc.gpsimd.dma_start(out=wg_sb, in_=wg_d)
            iden_sb = sb.tile([E, E], F32, name="iden_sb")
            nc.gpsimd.dma_start(out=iden_sb, in_=iden_d)

            # --- normalize wg rows: wg / max(||wg||, 1e-4) ---
            sq = sm.tile([E, RED], F32, name="sq")
            nc.vector.tensor_tensor(out=sq, in0=wg_sb, in1=wg_sb, op=ALU.mult)
            nrm = sm.tile([E, 1], F32, name="nrm")
            nc.vector.tensor_reduce(out=nrm, in_=sq, axis=mybir.AxisListType.X,
                                    op=ALU.add)
            nc.scalar.sqrt(nrm, nrm)
            nc.vector.tensor_scalar_max(out=nrm, in0=nrm, scalar1=1e-4)
            rcp = sm.tile([E, 1], F32, name="rcp")
            nc.vector.reciprocal(rcp, nrm)
            wgn = sm.tile([E, RED], F32, name="wgn")
            nc.vector.tensor_scalar_mul(out=wgn, in0=wg_sb, scalar1=rcp)

            # --- transpose wgn -> [RED, E] via PE ---
            pt = ps.tile([RED, E], F32, name="pt")
            nc.tensor.transpose(pt, wgn, iden_sb)
            wgnt = sm.tile([RED, E], F32, name="wgnt")
            nc.vector.tensor_copy(wgnt, pt)

            # --- redT [16, SHARD] = wrT.T @ xT  (fp32 exact) ---
            pr = ps.tile([RED, SHARD], F32, name="pr")
            for k in range(KC):
                nc.tensor.matmul(pr, wrt_sb[:, k * RED:(k + 1) * RED],
                                 xt_sb[:, k * SHARD:(k + 1) * SHARD],
                                 start=(k == 0), stop=(k == KC - 1))
            red_sb = sb.tile([RED, SHARD], F32, name="red_sb")
            nc.vector.tensor_copy(red_sb, pr)

            # --- per 128-token chunk: logits, score, argmax ---
            nchunk = SHARD // 128
            score_sb = sb.tile([128, nchunk], F32, name="score_sb")
            idx_sb = sb.tile([128, nchunk], I32, name="idx_sb")
            for j in range(nchunk):
                pl = psl.tile([128, E], F32, name="pl")
                nc.tensor.matmul(pl, red_sb[:, j * 128:(j + 1) * 128], wgnt,
                                 start=True, stop=True)
                lg = sm.tile([128, E], F32, name="lg")
                nc.vector.tensor_copy(lg, pl)
                mx8 = sm.tile([128, 8], F32, name="mx8")
                nc.vector.max(mx8, lg)
                ix8 = sm.tile([128, 8], U32, name="ix8")
                nc.vector.max_index(ix8, mx8, lg)
                nc.vector.tensor_copy(idx_sb[:, j:j + 1], ix8[:, 0:1])
                neg = sm.tile([128, 1], F32, name="neg")
                nc.vector.tensor_scalar_mul(out=neg, in0=mx8[:, 0:1],
                                            scalar1=-1.0)
                ex = sm.tile([128, E], F32, name="ex")
                ssum = sm.tile([128, 1], F32, name="ssum")
                nc.scalar.activation(ex, lg, AF.Exp, bias=neg, accum_out=ssum)
                nc.vector.reciprocal(score_sb[:, j:j + 1], ssum)

            nc.sync.dma_start(
                out=score_d.rearrange("(j p) -> p j", p=128), in_=score_sb)
            nc.sync.dma_start(
                out=idx_d.rearrange("(j p) -> p j", p=128), in_=idx_sb)

    nc.compile()
    return nc


# --------------------------------------------------------------------------
# Launch 2: expert FFN
# --------------------------------------------------------------------------
def _build_ffn():
    nc = bacc.Bacc("TRN2", target_bir_lowering=False, debug=False,
                   num_devices=NCORES)
    KC = C // 128    # 8
    MC = H // 128    # 32
    HB = 4           # H-chunks per w1 stream block
    NB = H // (HB * 128)  # 8 w1 stream blocks
    N0 = 512         # moving-dim split of CAP: 512 + 256
    N1 = CAP - N0

    xgt_d = nc.dram_tensor("xgt", [C, CAP], F32R, kind="ExternalInput").ap()
    w1_d = nc.dram_tensor("w1", [C, H], F32R, kind="ExternalInput").ap()
    w2_d = nc.dram_tensor("w2", [H, C], F32R, kind="ExternalInput").ap()
    b1_d = nc.dram_tensor("b1", [H], F32, kind="ExternalInput").ap()
    b2_d = nc.dram_tensor("b2", [C], F32, kind="ExternalInput").ap()
    sc_d = nc.dram_tensor("sc", [CAP], F32, kind="ExternalInput").ap()
    out_d = nc.dram_tensor("out", [C, CAP], F32, kind="ExternalOutput").ap()
    tot_d = nc.dram_tensor("tot", [1, 1], F32, kind="ExternalOutput").ap()

    HBW = 2          # H-chunks per w1 stream block (8 KB/partition)
    W1B = H // (HBW * 128)   # 16 w1 blocks
    KHALF = MC // 2  # 16 k-chunks per w2 half-block

    with tile.TileContext(nc) as tc:
        with tc.tile_pool(name="cst", bufs=1) as cst, \
             tc.tile_pool(name="xg", bufs=1) as xg, \
             tc.tile_pool(name="hh", bufs=1) as hh, \
             tc.tile_pool(name="ps", bufs=3, space="PSUM") as ps, \
             tc.tile_pool(name="pss", bufs=1, space="PSUM") as pss, \
             tc.tile_pool(name="ot", bufs=3) as ot, \
             tc.tile_pool(name="wp", bufs=4) as wp:
            # first w1 block as early as possible (PE ramp)
            w1blks = []
            blk = wp.tile([128, KC * HBW * 128], F32R, name="wblk")
            nc.sync.dma_start(
                out=blk.rearrange("p (k n) -> p k n", k=KC),
                in_=w1_d[:, 0:HBW * 128].rearrange("(k p) n -> p k n", p=128))
            w1blks.append(blk)

            xg_sb = xg.tile([128, KC * CAP], F32R, name="xg_sb")
            for k in range(KC):
                nc.gpsimd.dma_start(
                    out=xg_sb[:, k * CAP:(k + 1) * CAP],
                    in_=xgt_d[k * 128:(k + 1) * 128, :])

            b1_sb = cst.tile([128, MC], F32, name="b1_sb")
            nc.gpsimd.dma_start(out=b1_sb, in_=b1_d.rearrange("(m p) -> p m", p=128))
            b2_sb = cst.tile([128, KC], F32, name="b2_sb")
            nc.gpsimd.dma_start(out=b2_sb, in_=b2_d.rearrange("(m p) -> p m", p=128))
            sc_row = cst.tile([1, CAP], F32, name="sc_row")
            nc.gpsimd.dma_start(out=sc_row, in_=sc_d[None, :])
            ones_row = cst.tile([1, 128], F32, name="ones_row")
            nc.vector.memset(ones_row, 1.0)
            ones_col = cst.tile([128, 1], F32, name="ones_col")
            nc.vector.memset(ones_col, 1.0)

            # broadcast scores across partitions via K=1 matmul
            psb = ps.tile([128, CAP], F32, name="p")
            nc.tensor.matmul(psb[:, 0:N0], ones_row, sc_row[:, 0:N0],
                             start=True, stop=True)
            nc.tensor.matmul(psb[:, N0:CAP], ones_row, sc_row[:, N0:CAP],
                             start=True, stop=True)
            scb = cst.tile([128, CAP], F32, name="scb")
            nc.vector.tensor_copy(scb, psb)

            h_sb = hh.tile([128, MC * CAP], F32R, name="h_sb")

            # ---- fc1: hT = gelu(W1.T @ xgT + b1) ----
            for mb in range(W1B):
                if mb == 0:
                    w1blk = w1blks[0]
                else:
                    w1blk = wp.tile([128, KC * HBW * 128], F32R, name="wblk")
                    nc.sync.dma_start(
                        out=w1blk.rearrange("p (k n) -> p k n", k=KC),
                        in_=w1_d[:, mb * HBW * 128:(mb + 1) * HBW * 128]
                            .rearrange("(k p) n -> p k n", p=128))
                for mj in range(HBW):
                    m = mb * HBW + mj
                    p = ps.tile([128, CAP], F32, name="p")
                    for k in range(KC):
                        lhsT = w1blk[:, k * HBW * 128 + mj * 128:
                                     k * HBW * 128 + (mj + 1) * 128]
                        nc.tensor.matmul(p[:, 0:N0], lhsT,
                                         xg_sb[:, k * CAP:k * CAP + N0],
                                         start=(k == 0), stop=(k == KC - 1))
                        nc.tensor.matmul(p[:, N0:CAP], lhsT,
                                         xg_sb[:, k * CAP + N0:(k + 1) * CAP],
                                         start=(k == 0), stop=(k == KC - 1))
                    nc.scalar.activation(h_sb[:, m * CAP:(m + 1) * CAP], p,
                                         AF.Gelu, bias=b1_sb[:, m:m + 1])

            # ---- fc2: outT = (W2.T @ hT + b2) * score ----
            part_sb = cst.tile([128, KC], F32, name="part_sb")
            for m in range(KC):
                p = ps.tile([128, CAP], F32, name="p")
                for half in range(2):
                    w2blk = wp.tile([128, KHALF * 128], F32R, name="wblk")
                    nc.sync.dma_start(
                        out=w2blk.rearrange("p (k n) -> p k n", k=KHALF),
                        in_=w2_d[half * KHALF * 128:(half + 1) * KHALF * 128,
                                 m * 128:(m + 1) * 128]
                            .rearrange("(k p) n -> p k n", p=128))
                    for kk in range(KHALF):
                        k = half * KHALF + kk
                        lhsT = w2blk[:, kk * 128:(kk + 1) * 128]
                        nc.tensor.matmul(p[:, 0:N0], lhsT,
                                         h_sb[:, k * CAP:k * CAP + N0],
                                         start=(k == 0), stop=(k == MC - 1))
                        nc.tensor.matmul(p[:, N0:CAP], lhsT,
                                         h_sb[:, k * CAP + N0:(k + 1) * CAP],
                                         start=(k == 0), stop=(k == MC - 1))
                o = ot.tile([128, CAP], F32, name="o")
                nc.vector.scalar_tensor_tensor(
                    out=o, in0=p, scalar=b2_sb[:, m:m + 1], in1=scb,
                    op0=ALU.add, op1=ALU.mult,
                    accum_out=part_sb[:, m:m + 1])
                nc.scalar.dma_start(out=out_d[m * 128:(m + 1) * 128, :], in_=o)

            # ---- total = sum of partials ----
            rsum = cst.tile([128, 1], F32, name="rsum")
            nc.vector.tensor_reduce(out=rsum, in_=part_sb,
                                    axis=mybir.AxisListType.X, op=ALU.add)
            ptot = pss.tile([1, 1], F32, name="ptot")
            nc.tensor.matmul(ptot, ones_col, rsum, start=True, stop=True)
            tot_sb = cst.tile([1, 1], F32, name="tot_sb")
            nc.vector.tensor_copy(tot_sb, ptot)
            nc.sync.dma_start(out=tot_d, in_=tot_sb)

    nc.compile()
    return nc


def _get(name, builder):
    if name not in _cache:
        _cache[name] = builder()
    return _cache[name]


def _run(nc, in_maps, tag):
    trace = _trace_flag()
    if trace:
        res = run_bass_kernel_spmd(nc, in_maps, core_ids=list(range(NCORES)),
                                   trace=True)
        last_exec_ns[tag] = res.exec_time_ns
        return res.results
    res = run_bass_kernel_spmd(nc, in_maps, core_ids=list(range(NCORES)))
    return res.results


# --------------------------------------------------------------------------
# Host orchestration
# --------------------------------------------------------------------------
def kernel(x, wr, wg, w1, b1, w2, b2):
    x = np.ascontiguousarray(np.asarray(x, dtype=np.float32))
    wr = np.ascontiguousarray(np.asarray(wr, dtype=np.float32))
    wg = np.ascontiguousarray(np.asarray(wg, dtype=np.float32))
    w1 = np.ascontiguousarray(np.asarray(w1, dtype=np.float32))
    b1 = np.ascontiguousarray(np.asarray(b1, dtype=np.float32))
    w2 = np.ascontiguousarray(np.asarray(w2, dtype=np.float32))
    b2 = np.ascontiguousarray(np.asarray(b2, dtype=np.float32))

    B, T, _ = x.shape
    xf = x.reshape(S, C)
    xT = np.ascontiguousarray(xf.T)            # [C, S]
    wrt = np.ascontiguousarray(wr.T)           # [C, RED]
    iden = np.eye(E, dtype=np.float32)

    # ---- launch 1: gating (token-parallel shards) ----
    gate_nc = _get("gate", _build_gate)
    in_maps = [{
        "xt": np.ascontiguousarray(xT[:, c * SHARD:(c + 1) * SHARD]),
        "wrt": wrt, "wg": wg, "iden": iden,
    } for c in range(NCORES)]
    gres = _run(gate_nc, in_maps, "gate")
    idx = np.concatenate([gres[c]["idx"] for c in range(NCORES)])
    score = np.concatenate([gres[c]["score"] for c in range(NCORES)])

    # ---- host all-to-all dispatch ----
    token_lists = []
    ffn_maps = []
    for c in range(NCORES):
        tok = np.nonzero(idx == c)[0].astype(np.int64)
        n = tok.size
        assert n <= CAP, f"expert {c} got {n} tokens > CAP={CAP}"
        token_lists.append(tok)
        xgt = np.zeros((C, CAP), dtype=np.float32)
        xgt[:, :n] = xT[:, tok]
        sc = np.zeros(CAP, dtype=np.float32)
        sc[:n] = score[tok]
        ffn_maps.append({
            "xgt": xgt, "w1": w1[c], "w2": w2[c],
            "b1": b1[c], "b2": b2[c], "sc": sc,
        })

    # ---- launch 2: expert FFN ----
    ffn_nc = _get("ffn", _build_ffn)
    fres = _run(ffn_nc, ffn_maps, "ffn")

    # ---- combine ----
    out = np.empty((S, C), dtype=np.float32)
    total = np.float32(0.0)
    for c in range(NCORES):
        tok = token_lists[c]
        out[tok] = fres[c]["out"][:, :tok.size].T
        total = np.float32(total + fres[c]["tot"][0, 0])
    return out.reshape(B, T, C), total


# revision 9
# speedup vs baseline: 1.1416x; 1.1326x over previous
"""Trainium2 Bass kernel for nn_MoEBlock (top-1 MoE, E=8 experts).

Strategy (8 NeuronCores):
  Launch 1 (gate, token-parallel): each core computes fp32 gating for its
    512-token shard: red = x @ wr.T; logits = red @ normalize(wg).T;
    top-1 score (1/sum(exp(l - max))) and argmax index.
  Host: all-to-all dispatch — gather each expert's tokens (transposed,
    padded to CAP=768) for its owning core.
  Launch 2 (FFN, expert-parallel): core c holds expert c's weights in
    fp32r (TF32-like, full PE rate at N>=256): hT = gelu(W1.T @ xgT + b1),
    outT = (W2.T @ hT + b2) * score. Padded slots carry score=0 so they
    contribute exactly zero. Per-core partial sums reduce to the scalar
    total on-device.
  Host: scatter rows back by token id, add 8 partial sums.

Hardcoded for B=2, T=2048, C=1024, H=4096, E=8 (fixed problem shapes).
"""
import os
import sys

for _p in ("/root/.axon_site/_ro/trn_rl_repo", "/opt/trn_rl_repo"):
    if os.path.isdir(_p) and _p not in sys.path:
        sys.path.append(_p)

import numpy as np

import concourse.bacc as bacc
import concourse.mybir as mybir
import concourse.tile as tile
from concourse.bass_utils import run_bass_kernel_spmd

F32 = mybir.dt.float32
F32R = mybir.dt.float32r
I32 = mybir.dt.int32
U32 = mybir.dt.uint32
AF = mybir.ActivationFunctionType
ALU = mybir.AluOpType

S = 4096          # tokens
C = 1024          # model dim
H = 4096          # ffn dim
E = 8             # experts
RED = 16          # gate reduction dim
NCORES = 8
SHARD = S // NCORES   # tokens per core in the gate kernel
CAP = 768             # max tokens routed to one expert (actual max is 725)

# module-level cache: compiled programs + last exec times
_cache = {}
last_exec_ns = {"gate": None, "ffn": None}


def _trace_flag():
    return bool(int(os.environ.get("MOE_TRACE", "0")))


# --------------------------------------------------------------------------
# Launch 1: gating
# --------------------------------------------------------------------------
def _build_gate():
    nc = bacc.Bacc("TRN2", target_bir_lowering=False, debug=False,
                   num_devices=NCORES)
    KC = C // 128  # 8 K-chunks

    xt_d = nc.dram_tensor("xt", [C, SHARD], F32, kind="ExternalInput").ap()
    wrt_d = nc.dram_tensor("wrt", [C, RED], F32, kind="ExternalInput").ap()
    wg_d = nc.dram_tensor("wg", [E, RED], F32, kind="ExternalInput").ap()
    iden_d = nc.dram_tensor("iden", [E, E], F32, kind="ExternalInput").ap()
    # gout[p, j] = score of token j*128+p; gout[p, 4+j] = its argmax expert
    gout_d = nc.dram_tensor("gout", [128, 8], F32, kind="ExternalOutput").ap()

    with tile.TileContext(nc) as tc:
        with tc.tile_pool(name="sb", bufs=1) as sb, \
             tc.tile_pool(name="sm", bufs=2) as sm, \
             tc.tile_pool(name="ps", bufs=2, space="PSUM") as ps, \
             tc.tile_pool(name="psl", bufs=2, space="PSUM") as psl:
            wrt_sb = sb.tile([128, KC * RED], F32, name="wrt_sb")
            nc.gpsimd.dma_start(
                out=wrt_sb.rearrange("p (k n) -> p k n", k=KC),
                in_=wrt_d.rearrange("(k p) n -> p k n", p=128))
            xt_sb = sb.tile([128, KC * SHARD], F32, name="xt_sb")
            for k in range(KC):
                nc.sync.dma_start(
                    out=xt_sb[:, k * SHARD:(k + 1) * SHARD],
                    in_=xt_d[k * 128:(k + 1) * 128, :])
            wg_sb = sb.tile([E, RED], F32, name="wg_sb")
            nc.gpsimd.dma_start(out=wg_sb, in_=wg_d)
            iden_sb = sb.tile([E, E], F32, name="iden_sb")
            nc.gpsimd.dma_start(out=iden_sb, in_=iden_d)

            # --- normalize wg rows: wg / max(||wg||, 1e-4) ---
            sq = sm.tile([E, RED], F32, name="sq")
            nc.vector.tensor_tensor(out=sq, in0=wg_sb, in1=wg_sb, op=ALU.mult)
            nrm = sm.tile([E, 1], F32, name="nrm")
            nc.vector.tensor_reduce(out=nrm, in_=sq, axis=mybir.AxisListType.X,
                                    op=ALU.add)
            nc.scalar.sqrt(nrm, nrm)
            nc.vector.tensor_scalar_max(out=nrm, in0=nrm, scalar1=1e-4)
            rcp = sm.tile([E, 1], F32, name="rcp")
            nc.vector.reciprocal(rcp, nrm)
            wgn = sm.tile([E, RED], F32, name="wgn")
            nc.vector.tensor_scalar_mul(out=wgn, in0=wg_sb, scalar1=rcp)

            # --- transpose wgn -> [RED, E] via PE ---
            pt = ps.tile([RED, E], F32, name="pt")
            nc.tensor.transpose(pt, wgn, iden_sb)
            wgnt = sm.tile([RED, E], F32, name="wgnt")
            nc.vector.tensor_copy(wgnt, pt)

            # --- redT [16, SHARD] = wrT.T @ xT  (fp32 exact) ---
            pr = ps.tile([RED, SHARD], F32, name="pr")
            for k in range(KC):
                nc.tensor.matmul(pr, wrt_sb[:, k * RED:(k + 1) * RED],
                                 xt_sb[:, k * SHARD:(k + 1) * SHARD],
                                 start=(k == 0), stop=(k == KC - 1))
            red_sb = sb.tile([RED, SHARD], F32, name="red_sb")
            nc.vector.tensor_copy(red_sb, pr)

            # --- per 128-token chunk: logits, score, argmax ---
            nchunk = SHARD // 128
            gout_sb = sb.tile([128, 2 * nchunk], F32, name="gout_sb")
            for j in range(nchunk):
                pl = psl.tile([128, E], F32, name="pl")
                nc.tensor.matmul(pl, red_sb[:, j * 128:(j + 1) * 128], wgnt,
                                 start=True, stop=True)
                lg = sm.tile([128, E], F32, name="lg")
                nc.vector.tensor_copy(lg, pl)
                mx8 = sm.tile([128, 8], F32, name="mx8")
                nc.vector.max(mx8, lg)
                ix8 = sm.tile([128, 8], U32, name="ix8")
                nc.vector.max_index(ix8, mx8, lg)
                nc.vector.tensor_copy(gout_sb[:, nchunk + j:nchunk + j + 1],
                                      ix8[:, 0:1])
                neg = sm.tile([128, 1], F32, name="neg")
                nc.vector.tensor_scalar_mul(out=neg, in0=mx8[:, 0:1],
                                            scalar1=-1.0)
                ex = sm.tile([128, E], F32, name="ex")
                ssum = sm.tile([128, 1], F32, name="ssum")
                nc.scalar.activation(ex, lg, AF.Exp, bias=neg, accum_out=ssum)
                nc.vector.reciprocal(gout_sb[:, j:j + 1], ssum)

            nc.sync.dma_start(out=gout_d, in_=gout_sb)

    nc.compile()
    return nc


# --------------------------------------------------------------------------
# Launch 2: expert FFN
# --------------------------------------------------------------------------
def _build_ffn():
    nc = bacc.Bacc("TRN2", target_bir_lowering=False, debug=False,
                   num_devices=NCORES)
    KC = C // 128    # 8
    MC = H // 128    # 32
    HB = 4           # H-chunks per w1 stream block
    NB = H // (HB * 128)  # 8 w1 stream blocks
    N0 = 512         # moving-dim split of CAP: 512 + 256
    N1 = CAP - N0

    xgt_d = nc.dram_tensor("xgt", [C, CAP], F32R, kind="ExternalInput").ap()
    w1_d = nc.dram_tensor("w1", [C, H], F32R, kind="ExternalInput").ap()
    w2_d = nc.dram_tensor("w2", [H, C], F32R, kind="ExternalInput").ap()
    b1_d = nc.dram_tensor("b1", [H], F32, kind="ExternalInput").ap()
    b2_d = nc.dram_tensor("b2", [C], F32, kind="ExternalInput").ap()
    sc_d = nc.dram_tensor("sc", [CAP], F32, kind="ExternalInput").ap()
    out_d = nc.dram_tensor("out", [C, CAP], F32, kind="ExternalOutput").ap()
    tot_d = nc.dram_tensor("tot", [1, 1], F32, kind="ExternalOutput").ap()

    HBW = 2          # H-chunks per w1 stream block (8 KB/partition)
    W1B = H // (HBW * 128)   # 16 w1 blocks
    KHALF = MC // 2  # 16 k-chunks per w2 half-block

    with tile.TileContext(nc) as tc:
        with tc.tile_pool(name="cst", bufs=1) as cst, \
             tc.tile_pool(name="xg", bufs=1) as xg, \
             tc.tile_pool(name="hh", bufs=1) as hh, \
             tc.tile_pool(name="ps", bufs=3, space="PSUM") as ps, \
             tc.tile_pool(name="pss", bufs=1, space="PSUM") as pss, \
             tc.tile_pool(name="ot", bufs=3) as ot, \
             tc.tile_pool(name="wp", bufs=4) as wp:
            # first w1 block as early as possible (PE ramp)
            w1blks = []
            blk = wp.tile([128, KC * HBW * 128], F32R, name="wblk")
            nc.sync.dma_start(
                out=blk.rearrange("p (k n) -> p k n", k=KC),
                in_=w1_d[:, 0:HBW * 128].rearrange("(k p) n -> p k n", p=128))
            w1blks.append(blk)

            xg_sb = xg.tile([128, KC * CAP], F32R, name="xg_sb")
            for k in range(KC):
                nc.gpsimd.dma_start(
                    out=xg_sb[:, k * CAP:(k + 1) * CAP],
                    in_=xgt_d[k * 128:(k + 1) * 128, :])

            b1_sb = cst.tile([128, MC], F32, name="b1_sb")
            nc.gpsimd.dma_start(out=b1_sb, in_=b1_d.rearrange("(m p) -> p m", p=128))
            b2_sb = cst.tile([128, KC], F32, name="b2_sb")
            nc.gpsimd.dma_start(out=b2_sb, in_=b2_d.rearrange("(m p) -> p m", p=128))
            sc_row = cst.tile([1, CAP], F32, name="sc_row")
            nc.gpsimd.dma_start(out=sc_row, in_=sc_d[None, :])
            ones_row = cst.tile([1, 128], F32, name="ones_row")
            nc.vector.memset(ones_row, 1.0)
            ones_col = cst.tile([128, 1], F32, name="ones_col")
            nc.vector.memset(ones_col, 1.0)

            h_sb = hh.tile([128, MC * CAP], F32R, name="h_sb")

            # ---- fc1: hT = gelu(W1.T @ xgT + b1) ----
            for mb in range(W1B):
                if mb == 0:
                    w1blk = w1blks[0]
                else:
                    w1blk = wp.tile([128, KC * HBW * 128], F32R, name="wblk")
                    nc.sync.dma_start(
                        out=w1blk.rearrange("p (k n) -> p k n", k=KC),
                        in_=w1_d[:, mb * HBW * 128:(mb + 1) * HBW * 128]
                            .rearrange("(k p) n -> p k n", p=128))
                for mj in range(HBW):
                    m = mb * HBW + mj
                    p = ps.tile([128, CAP], F32, name="p")
                    for k in range(KC):
                        lhsT = w1blk[:, k * HBW * 128 + mj * 128:
                                     k * HBW * 128 + (mj + 1) * 128]
                        nc.tensor.matmul(p[:, 0:N0], lhsT,
                                         xg_sb[:, k * CAP:k * CAP + N0],
                                         start=(k == 0), stop=(k == KC - 1))
                        nc.tensor.matmul(p[:, N0:CAP], lhsT,
                                         xg_sb[:, k * CAP + N0:(k + 1) * CAP],
                                         start=(k == 0), stop=(k == KC - 1))
                    nc.scalar.activation(h_sb[:, m * CAP:(m + 1) * CAP], p,
                                         AF.Gelu, bias=b1_sb[:, m:m + 1])

            # broadcast scores across partitions via K=1 matmul
            # (emitted after fc1 so it doesn't block the PE FIFO at startup)
            psb = ps.tile([128, CAP], F32, name="p")
            nc.tensor.matmul(psb[:, 0:N0], ones_row, sc_row[:, 0:N0],
                             start=True, stop=True)
            nc.tensor.matmul(psb[:, N0:CAP], ones_row, sc_row[:, N0:CAP],
                             start=True, stop=True)
            scb = cst.tile([128, CAP], F32, name="scb")
            nc.vector.tensor_copy(scb, psb)

            # ---- fc2: outT = (W2.T @ hT + b2) * score ----
            part_sb = cst.tile([128, KC], F32, name="part_sb")
            for m in range(KC):
                p = ps.tile([128, CAP], F32, name="p")
                for half in range(2):
                    w2blk = wp.tile([128, KHALF * 128], F32R, name="wblk")
                    nc.sync.dma_start(
                        out=w2blk.rearrange("p (k n) -> p k n", k=KHALF),
                        in_=w2_d[half * KHALF * 128:(half + 1) * KHALF * 128,
                                 m * 128:(m + 1) * 128]
                            .rearrange("(k p) n -> p k n", p=128))
                    for kk in range(KHALF):
                        k = half * KHALF + kk
                        lhsT = w2blk[:, kk * 128:(kk + 1) * 128]
                        nc.tensor.matmul(p[:, 0:N0], lhsT,
                                         h_sb[:, k * CAP:k * CAP + N0],
                                         start=(k == 0), stop=(k == MC - 1))
                        nc.tensor.matmul(p[:, N0:CAP], lhsT,
                                         h_sb[:, k * CAP + N0:(k + 1) * CAP],
                                         start=(k == 0), stop=(k == MC - 1))
                o = ot.tile([128, CAP], F32, name="o")
                nc.vector.scalar_tensor_tensor(
                    out=o, in0=p, scalar=b2_sb[:, m:m + 1], in1=scb,
                    op0=ALU.add, op1=ALU.mult,
                    accum_out=part_sb[:, m:m + 1])
                nc.scalar.dma_start(out=out_d[m * 128:(m + 1) * 128, :], in_=o)

            # ---- total = sum of partials ----
            rsum = cst.tile([128, 1], F32, name="rsum")
            nc.vector.tensor_reduce(out=rsum, in_=part_sb,
                                    axis=mybir.AxisListType.X, op=ALU.add)
            ptot = pss.tile([1, 1], F32, name="ptot")
            nc.tensor.matmul(ptot, ones_col, rsum, start=True, stop=True)
            tot_sb = cst.tile([1, 1], F32, name="tot_sb")
            nc.vector.tensor_copy(tot_sb, ptot)
            nc.sync.dma_start(out=tot_d, in_=tot_sb)

    nc.compile()
    return nc


def _get(name, builder):
    if name not in _cache:
        _cache[name] = builder()
    return _cache[name]


def _run(nc, in_maps, tag):
    trace = _trace_flag()
    if trace:
        res = run_bass_kernel_spmd(nc, in_maps, core_ids=list(range(NCORES)),
                                   trace=True)
        last_exec_ns[tag] = res.exec_time_ns
        return res.results
    res = run_bass_kernel_spmd(nc, in_maps, core_ids=list(range(NCORES)))
    return res.results


# --------------------------------------------------------------------------
# Host orchestration
# --------------------------------------------------------------------------
def kernel(x, wr, wg, w1, b1, w2, b2):
    x = np.ascontiguousarray(np.asarray(x, dtype=np.float32))
    wr = np.ascontiguousarray(np.asarray(wr, dtype=np.float32))
    wg = np.ascontiguousarray(np.asarray(wg, dtype=np.float32))
    w1 = np.ascontiguousarray(np.asarray(w1, dtype=np.float32))
    b1 = np.ascontiguousarray(np.asarray(b1, dtype=np.float32))
    w2 = np.ascontiguousarray(np.asarray(w2, dtype=np.float32))
    b2 = np.ascontiguousarray(np.asarray(b2, dtype=np.float32))

    B, T, _ = x.shape
    xf = x.reshape(S, C)
    xT = np.ascontiguousarray(xf.T)            # [C, S]
    wrt = np.ascontiguousarray(wr.T)           # [C, RED]
    iden = np.eye(E, dtype=np.float32)

    # ---- launch 1: gating (token-parallel shards) ----
    gate_nc = _get("gate", _build_gate)
    in_maps = [{
        "xt": np.ascontiguousarray(xT[:, c * SHARD:(c + 1) * SHARD]),
        "wrt": wrt, "wg": wg, "iden": iden,
    } for c in range(NCORES)]
    gres = _run(gate_nc, in_maps, "gate")
    nch = SHARD // 128
    score = np.concatenate(
        [gres[c]["gout"][:, :nch].T.ravel() for c in range(NCORES)])
    idx = np.concatenate(
        [gres[c]["gout"][:, nch:].T.ravel() for c in range(NCORES)]
    ).astype(np.int64)

    # ---- host all-to-all dispatch ----
    token_lists = []
    ffn_maps = []
    for c in range(NCORES):
        tok = np.nonzero(idx == c)[0].astype(np.int64)
        n = tok.size
        assert n <= CAP, f"expert {c} got {n} tokens > CAP={CAP}"
        token_lists.append(tok)
        xgt = np.zeros((C, CAP), dtype=np.float32)
        xgt[:, :n] = xT[:, tok]
        sc = np.zeros(CAP, dtype=np.float32)
        sc[:n] = score[tok]
        ffn_maps.append({
            "xgt": xgt, "w1": w1[c], "w2": w2[c],
            "b1": b1[c], "b2": b2[c], "sc": sc,
        })

    # ---- launch 2: expert FFN ----
    ffn_nc = _get("ffn", _build_ffn)
    fres = _run(ffn_nc, ffn_maps, "ffn")

    # ---- combine ----
    out = np.empty((S, C), dtype=np.float32)
    total = np.float32(0.0)
    for c in range(NCORES):
        tok = token_lists[c]
        out[tok] = fres[c]["out"][:, :tok.size].T
        total = np.float32(total + fres[c]["tot"][0, 0])
    return out.reshape(B, T, C), total
